# revision 24
# baseline (speedup 1.0000x reference)
"""
MultiHeadCrossAttention Trainium2 kernel (Bass/Tile), data-parallel over batch
on 8 NeuronCores.

Reference computation (per batch row b):
    Q = text @ Wq.T + bq          [B, 1024] -> [B, 8, 128]
    K = image @ Wk.T + bk         [B, 2048] -> [B, 8, 128]
    V = image @ Wv.T + bv         [B, 2048] -> [B, 8, 128]
    scores[b,h,g] = Q[b,h,:].K[b,g,:] / sqrt(128)
    attn = softmax_g(scores)
    attended[b,h,:] = sum_g attn[b,h,g] V[b,g,:]
    y = LayerNorm(text + attended) * gamma + beta

v2 design (per core, B_loc = 2048 batch rows):
  - Projections on the PE in fp8e4 with DoubleRow perf mode (2 contraction
    k-tiles of 128 packed per matmul -> 0.5 cycles/row).  Host pre-casts
    X^T slabs (natural scale) and W^T (scaled x128) to e4m3; biases ride a
    K=1 fp16 matmul pre-scaled x128; the PSUM->SBUF ACT copy applies 1/128,
    restoring natural units in fp16.
  - Attention entirely in batch-on-partition layout on the DVE: broadcast
    product + dense binary add-tree for the d-reduction (scores) and the
    g-reduction (attend).  These are the critical-path ops (TT runs at
    2 elem/cycle fp16; nothing else on the chip multiplies tensors faster).
  - LayerNorm stats via ACT accumulate (Square+accum, Identity+accum), the
    rsqrt via Ln/Exp (one ACT table), normalize+shift as a single ACT op
    writing the fp16 output tile directly.  gamma/beta are identity in this
    problem (ones/zeros) and are applied on host in the (never-taken)
    general case, so the device kernel skips them.
"""

import functools
import sys

import numpy as np

sys.path.insert(0, "/opt/trn_rl_repo")

import concourse.bass as bass  # noqa: E402
import concourse.tile as tile  # noqa: E402
from concourse import bacc, bass_utils, mybir  # noqa: E402

import ml_dtypes  # noqa: E402


def _patch_act_tables():
    """Force every activation we use (Exp/Ln/Square/Copy/Identity) to resolve
    to the one table set that holds them all (natural_log_exp_and_others), so
    bacc emits a single ACT table load instead of thrashing (1.28us/swap)."""
    import concourse.hw_specs as hw_specs

    orig = hw_specs.get_activation_tables
    if getattr(orig, "_mhca_patched", False):
        return

    A = mybir.ActivationFunctionType
    KEEP = "natural_log_exp_and_others"

    @functools.cache
    def patched(arch):
        tabs = {k: set(v) for k, v in orig(arch).items()}
        for k, s in tabs.items():
            if k != KEEP:
                for f in (A.Exp, A.Ln, A.Square, A.Copy, A.Identity):
                    s.discard(f)
        return tabs

    patched._mhca_patched = True
    hw_specs.get_activation_tables = patched
    import concourse.bass_interp as _bi

    _bi.get_activation_tables = patched
    bacc.get_activation_tables = patched


_patch_act_tables()

# Problem constants (hardcoded per contest contract)
B = 16384
N_CORES = 8
B_LOC = B // N_CORES  # 2048
TEXT_DIM = 1024
IMAGE_DIM = 2048
H = 8
HD = 128
NTC2 = TEXT_DIM // 256  # 4 DoubleRow k-chunks (256 each)
NIC2 = IMAGE_DIM // 256  # 8 DoubleRow k-chunks

BT = 128  # batch tile (partition dim)
PHASE = 2  # batch tiles per X^T slab load

F8 = mybir.dt.float8e4
F16 = mybir.dt.float16
F32 = mybir.dt.float32
NP_F8 = ml_dtypes.float8_e4m3

S_W = 128.0  # fp8 weight pre-scale (power of 2; undone in the psum copy)

INV_SQRT_HD = 1.0 / np.sqrt(128.0)

# V feature permutation: f' = d*8 + g for original f = g*128 + d, i.e. V is
# stored with the 8 head values of each hidden position adjacent, so the
# attend product / g-reduction reads contiguous 8-element runs.
_d, _g = np.meshgrid(np.arange(128), np.arange(8), indexing="ij")
V_PERM = (_g * 128 + _d).reshape(-1)  # V_PERM[f'] = original f

DR = mybir.MatmulPerfMode.DoubleRow


def build_bass(b_loc: int = B_LOC) -> bass.Bass:
    nt = b_loc // BT
    phase = min(PHASE, nt)
    bw = phase * BT

    nc = bacc.Bacc(trn_type="TRN2", debug=False, name="mhca_dp", num_swdge_queues=4)

    # ---- DRAM I/O ----
    # X^T slabs in DoubleRow layout: d_in = c*256 + i*128 + p
    text_t8 = nc.dram_tensor("text_t8", [128, NTC2, 2, b_loc], F8, kind="ExternalInput")
    image_t8 = nc.dram_tensor("image_t8", [128, NIC2, 2, b_loc], F8, kind="ExternalInput")
    text16 = nc.dram_tensor("text16", [b_loc, TEXT_DIM], F16, kind="ExternalInput")
    wq8 = nc.dram_tensor("wq8", [128, NTC2, 2, TEXT_DIM], F8, kind="ExternalInput")
    wk8 = nc.dram_tensor("wk8", [128, NIC2, 2, TEXT_DIM], F8, kind="ExternalInput")
    wv8 = nc.dram_tensor("wv8", [128, NIC2, 2, TEXT_DIM], F8, kind="ExternalInput")
    bq = nc.dram_tensor("bq", [1, TEXT_DIM], F16, kind="ExternalInput")
    bk = nc.dram_tensor("bk", [1, TEXT_DIM], F16, kind="ExternalInput")
    bv = nc.dram_tensor("bv", [1, TEXT_DIM], F16, kind="ExternalInput")
    y = nc.dram_tensor("y", [b_loc, TEXT_DIM], F16, kind="ExternalOutput")

    with tile.TileContext(nc) as tc:
        _body(nc, tc, locals(), nt=nt, phase=phase, bw=bw)
    nc.compile()
    return nc


def _ap(t: bass.AP, dims) -> bass.AP:
    """Raw AP on an SBUF tile: keep its partition dim, custom free dims."""
    return bass.AP(tensor=t.tensor, offset=t.offset, ap=[list(t.ap[0])] + [list(d) for d in dims])


def _body(nc: bass.Bass, tc: tile.TileContext, io: dict, *, nt: int, phase: int, bw: int):
    text_t8, image_t8, text16 = io["text_t8"], io["image_t8"], io["text16"]
    wq8, wk8, wv8 = io["wq8"], io["wk8"], io["wv8"]
    bq, bk, bv = io["bq"], io["bk"], io["bv"]
    y = io["y"]

    import contextlib

    ctx = contextlib.ExitStack()
    with ctx:
        consts = ctx.enter_context(tc.tile_pool(name="consts", bufs=1))
        slabs = ctx.enter_context(tc.tile_pool(name="slabs", bufs=3))
        qkv = ctx.enter_context(tc.tile_pool(name="qkv", bufs=3))
        work = ctx.enter_context(tc.tile_pool(name="work", bufs=2))
        prods = ctx.enter_context(tc.tile_pool(name="prods", bufs=2))
        scr2p = ctx.enter_context(tc.tile_pool(name="scr2p", bufs=2))
        outs = ctx.enter_context(tc.tile_pool(name="outs", bufs=2))
        small = ctx.enter_context(tc.tile_pool(name="small", bufs=3))
        psum = ctx.enter_context(tc.tile_pool(name="psum", bufs=4, space="PSUM"))

        # ---- constants / weights (fp8, host-prepared) ----
        w8_q = consts.tile([128, NTC2, 2, TEXT_DIM], F8)
        w8_k = consts.tile([128, NIC2, 2, TEXT_DIM], F8)
        w8_v = consts.tile([128, NIC2, 2, TEXT_DIM], F8)
        # weight loads spread over the two HWDGE channels (sync / scalar),
        # ordered so tile 0's consumption order (Q -> K -> V) is also the
        # landing order.  Keeping them off the gpsimd SWDGE rings matters:
        # those also carry the X^T slabs, and a 2MB weight load queued behind
        # a slab stalls the first tiles by tens of us.
        wload = [
            (nc.sync, w8_q, wq8, 0, 4),
            (nc.scalar, w8_k, wk8, 0, 4),
            (nc.gpsimd, w8_k, wk8, 4, 4),
            (nc.gpsimd, w8_v, wv8, 0, 4),
            (nc.gpsimd, w8_v, wv8, 4, 4),
        ]

        b16 = consts.tile([1, 3, TEXT_DIM], F16)
        nc.gpsimd.dma_start(out=b16[:, 0, :], in_=bq[:])
        nc.gpsimd.dma_start(out=b16[:, 1, :], in_=bk[:])
        nc.gpsimd.dma_start(out=b16[:, 2, :], in_=bv[:])

        ones16 = consts.tile([1, 128], F16)
        nc.vector.memset(ones16, 1.0)

        # PE warm-up: the tensor engine only reaches max clock after ~3us of
        # continuous execution.  Burn ~5us of dependency-free K=1 matmuls at
        # t=0 (while the weights/slabs are still in flight) so the first real
        # projections run on a hot PE instead of at the cold p-state.
        junk = psum.tile([128, TEXT_DIM], F32, tag="psum")
        for _ in range(40):
            nc.tensor.matmul(junk[:, 0:128], lhsT=ones16, rhs=ones16,
                             start=True, stop=True)

        eps_sb = consts.tile([128, 1], F32)
        nc.vector.memset(eps_sb, 1e-5)
        sq16 = consts.tile([128, TEXT_DIM], F16)  # ACT-accum scratch output

        # ---------------- 3-stage software pipeline ----------------
        # stage A (iter j):   projections + psum copies + scores + exp
        # stage B (iter j+1): softmax weights + attend + residual add
        # stage C (iter j+2): layernorm + store
        # This keeps the DVE FIFO free of head-of-line stalls: every DVE op
        # emitted only depends on work issued at least one iteration earlier.

        def project(xt, w8, nchunks2, bias_idx, bs):
            pt = psum.tile([128, TEXT_DIM], F32, tag="psum")
            for f in range(2):
                half = pt[:, f * 512 : (f + 1) * 512]
                for c in range(nchunks2):
                    nc.tensor.matmul(
                        half,
                        lhsT=xt[:, c, :, bs],
                        rhs=w8[:, c, :, f * 512 : (f + 1) * 512],
                        start=(c == 0),
                        stop=False,
                        perf_mode=DR,
                    )
                nc.tensor.matmul(
                    half,
                    lhsT=ones16,
                    rhs=b16[:, bias_idx, f * 512 : (f + 1) * 512],
                    start=False,
                    stop=True,
                )
            return pt

        def stage_a(it, xt_text, xt_img, bs):
            row0 = it * BT
            text_sb = work.tile([128, TEXT_DIM], F16, tag="text_sb")
            nc.sync.dma_start(out=text_sb, in_=text16[row0 : row0 + BT, :])

            qp = project(xt_text, w8_q, NTC2, 0, bs)
            kp = project(xt_img, w8_k, NIC2, 1, bs)
            vp = project(xt_img, w8_v, NIC2, 2, bs)

            # PSUM -> SBUF fp16 copies (ACT), undoing the x128 weight scale.
            # Q/K first — the DVE scores product needs them next; the V copy
            # is emitted after the exp (below) so the softmax-denominator
            # dependency is never queued behind it.  Wv/bv are host-permuted
            # to the [d][g] attend layout, so all copies are contiguous.
            q16 = qkv.tile([128, TEXT_DIM], F16, tag="q16")
            k16 = qkv.tile([128, TEXT_DIM], F16, tag="k16")
            vt16 = qkv.tile([128, TEXT_DIM], F16, tag="vt16")
            nc.scalar.mul(q16, qp, 1.0 / S_W)
            nc.scalar.mul(k16, kp, 1.0 / S_W)

            # scores: prod[b, h, g, d] = Q[b,h,d] * K[b,g,d]
            prod = prods.tile([128, H * H * HD], F16, tag="prod")
            scr2 = scr2p.tile([128, H * H * HD // 2], F16, tag="scr2")
            nc.vector.tensor_tensor(
                out=prod[:].rearrange("p (h g d) -> p h g d", h=H, g=H),
                in0=_ap(q16, [[128, 8], [0, 8], [1, 128]]),
                in1=_ap(k16, [[0, 8], [128, 8], [1, 128]]),
                op=mybir.AluOpType.mult,
            )
            # d-reduction: binary TT-add tree with dense (compacted) outputs
            # ping-ponging between prod and scr2 — sparse in-place outputs hit
            # a DVE slow path — then one tensor_reduce of the remaining 8.
            cur, nxt = prod, scr2
            d = HD
            while d > 8:
                nc.vector.tensor_tensor(
                    out=_ap(nxt, [[d // 2, H * H], [1, d // 2]]),
                    in0=_ap(cur, [[d, H * H], [1, d // 2]]),
                    in1=bass.AP(tensor=cur.tensor, offset=cur.offset + d // 2,
                                ap=[list(cur.ap[0]), [d, H * H], [1, d // 2]]),
                    op=mybir.AluOpType.add,
                )
                cur, nxt = nxt, cur
                d //= 2
            s16 = small.tile([128, H * H], F16, tag="s16")
            with nc.allow_low_precision("fp16 scores; DVE ALU accumulates fp32"):
                nc.vector.tensor_reduce(
                    out=s16,
                    in_=_ap(cur, [[8, H * H], [1, 8]]),
                    axis=mybir.AxisListType.X,
                    op=mybir.AluOpType.add,
                )
            e16 = small.tile([128, H * H], F16, tag="e16")
            nc.scalar.activation(
                out=e16, in_=s16,
                func=mybir.ActivationFunctionType.Exp,
                scale=float(INV_SQRT_HD),
            )
            # V copy after the exp: stage_b consumes it one iteration later
            nc.scalar.mul(vt16, vp, 1.0 / S_W)
            return dict(it=it, text_sb=text_sb, vt16=vt16, e16=e16, prod=prod, scr2=scr2)

        def stage_b(t):
            e16, vt16, prod, scr2 = t["e16"], t["vt16"], t["prod"], t["scr2"]
            den = small.tile([128, H], F32, tag="den")
            nc.vector.tensor_reduce(
                out=den,
                in_=e16[:].rearrange("p (h g) -> p h g", h=H),
                axis=mybir.AxisListType.X,
                op=mybir.AluOpType.add,
            )
            rden = small.tile([128, H], F32, tag="rden")
            nc.vector.reciprocal(out=rden, in_=den)  # = 1 / sum_g exp
            a16 = small.tile([128, H * H], F16, tag="a16")
            nc.vector.tensor_tensor(
                out=a16[:].rearrange("p (h g) -> p h g", h=H),
                in0=e16[:].rearrange("p (h g) -> p h g", h=H),
                in1=_ap(rden, [[1, 8], [0, 8]]),
                op=mybir.AluOpType.mult,
            )
            # attend, A/B-split over g so every reduction stream is long-run:
            #   prodA[b, h, d, g<4]  = Vperm[b, d*8+g]   * A[b,h,g]
            #   prodB[b, h, d, g'<4] = Vperm[b, d*8+4+g'] * A[b,h,4+g']
            # (V stream first: long contiguous runs on the src0 port)
            half = H * HD * 4  # 4096
            for off in (0, 4):
                nc.vector.tensor_tensor(
                    out=bass.AP(tensor=prod.tensor, offset=prod.offset + (off // 4) * half,
                                ap=[list(prod.ap[0]), [4, H * HD], [1, 4]]),
                    in0=bass.AP(tensor=vt16.tensor, offset=vt16.offset + off,
                                ap=[list(vt16.ap[0]), [0, 8], [8, 128], [1, 4]]),
                    in1=bass.AP(tensor=a16.tensor, offset=a16.offset + off,
                                ap=[list(a16.ap[0]), [8, 8], [0, 128], [1, 4]]),
                    op=mybir.AluOpType.mult,
                )
            # g-reduction: L1 = A-half + B-half (both fully contiguous)
            nc.vector.tensor_tensor(
                out=_ap(scr2, [[1, half]]),
                in0=_ap(prod, [[1, half]]),
                in1=bass.AP(tensor=prod.tensor, offset=prod.offset + half,
                            ap=[list(prod.ap[0]), [1, half]]),
                op=mybir.AluOpType.add,
            )
            # L2: pairs (g, g+2) within each [h,d] group of 4
            nc.vector.tensor_tensor(
                out=_ap(prod, [[2, H * HD], [1, 2]]),
                in0=_ap(scr2, [[4, H * HD], [1, 2]]),
                in1=bass.AP(tensor=scr2.tensor, offset=scr2.offset + 2,
                            ap=[list(scr2.ap[0]), [4, H * HD], [1, 2]]),
                op=mybir.AluOpType.add,
            )
            att16 = work.tile([128, TEXT_DIM], F16, tag="att16")
            nc.vector.tensor_tensor(
                out=att16,
                in0=_ap(prod, [[2, H * HD]]),
                in1=bass.AP(tensor=prod.tensor, offset=prod.offset + 1,
                            ap=[list(prod.ap[0]), [2, H * HD]]),
                op=mybir.AluOpType.add,
            )
            # residual add stays on the DVE: contiguous fp16 at 2 elem/cycle
            # beats the gpsimd software add by ~10x and unblocks the pipeline.
            x = work.tile([128, TEXT_DIM], F16, tag="x")
            nc.vector.tensor_tensor(
                out=x, in0=t["text_sb"], in1=att16, op=mybir.AluOpType.add
            )
            t["x"] = x

        def stage_c(t):
            x = t["x"]
            row0 = t["it"] * BT
            A = mybir.ActivationFunctionType
            # LN stats on ACT: ssq = sum x^2, sx = sum x (sq16 is a dump tile)
            ssq = small.tile([128, 1], F32, tag="ssq")
            sx = small.tile([128, 1], F32, tag="sx")
            nc.scalar.activation(out=sq16, in_=x, func=A.Square, accum_out=ssq)
            nc.scalar.activation(out=sq16, in_=x, func=A.Identity, accum_out=sx)
            # [128,1] scalar math on the (otherwise idle) gpsimd:
            # mu = sx/D ; m2 = -mu^2 ; varp = ssq/D + eps - mu^2
            mu = small.tile([128, 1], F32, tag="mu")
            nc.gpsimd.tensor_scalar(
                out=mu, in0=sx, scalar1=1.0 / TEXT_DIM, scalar2=1.0,
                op0=mybir.AluOpType.mult, op1=mybir.AluOpType.mult,
            )
            m2 = small.tile([128, 1], F32, tag="m2")
            nc.gpsimd.tensor_scalar(
                out=m2, in0=mu, scalar1=mu, scalar2=-1.0,
                op0=mybir.AluOpType.mult, op1=mybir.AluOpType.mult,
            )
            varp = small.tile([128, 1], F32, tag="varp")
            nc.gpsimd.tensor_scalar(
                out=varp, in0=ssq, scalar1=1.0 / TEXT_DIM, scalar2=m2,
                op0=mybir.AluOpType.mult, op1=mybir.AluOpType.add,
            )
            # rs = 1/sqrt(var+eps) = exp(-0.5*ln(var+eps)); Ln and Exp live in
            # the same ACT table (natural_log_exp_and_others), Sqrt does not.
            # eps rides the Ln bias.
            lnv = small.tile([128, 1], F32, tag="lnv")
            nc.scalar.activation(out=lnv, in_=varp, func=A.Ln, bias=eps_sb)
            rs = small.tile([128, 1], F32, tag="rs")
            nc.scalar.activation(out=rs, in_=lnv, func=A.Exp, scale=-0.5)
            nmr = small.tile([128, 1], F32, tag="nmr")
            nc.gpsimd.tensor_scalar(
                out=nmr, in0=mu, scalar1=rs, scalar2=-1.0,
                op0=mybir.AluOpType.mult, op1=mybir.AluOpType.mult,
            )
            # y = x*rs - mu*rs on ACT — keeps the wall engine (DVE) clean;
            # this sits at the end of ACT's queue where nothing urgent follows.
            # (gamma/beta are identity; host applies the general case)
            y16 = outs.tile([128, TEXT_DIM], F16, tag="y16")
            nc.scalar.activation(out=y16, in_=x, func=A.Identity, scale=rs, bias=nmr)
            nc.gpsimd.dma_start(out=y[row0 : row0 + BT, :], in_=y16)

        def load_slab(p):
            b0 = p * bw
            xt_t = slabs.tile([128, NTC2, 2, bw], F8, tag="xt_text")
            xt_i = slabs.tile([128, NIC2, 2, bw], F8, tag="xt_img")
            nc.gpsimd.dma_start(out=xt_t, in_=text_t8[:, :, :, b0 : b0 + bw])
            nc.gpsimd.dma_start(out=xt_i, in_=image_t8[:, :, :, b0 : b0 + bw])
            return xt_t, xt_i

        pend = []
        n_phases = (nt + phase - 1) // phase
        slab_next = load_slab(0)
        for ph in range(n_phases):
            xt_text, xt_img = slab_next
            if ph == 0:
                # weights after the first slab so tile 0 lhsT lands first
                for eng, w8, wr, c0, cn in wload:
                    eng.dma_start(
                        out=w8[:, c0 : c0 + cn], in_=wr[:, c0 : c0 + cn]
                    )
            if ph + 1 < n_phases:
                # prefetch the next phase's slab before this phase's tiles so
                # a phase boundary never stalls on the X^T load
                slab_next = load_slab(ph + 1)

            for j in range(phase):
                it = ph * phase + j
                if it >= nt:
                    break
                # stage_a(j) BEFORE stage_b(j-1): the DVE then grinds through
                # prod1/tree(j) between s16(j-1)'s completion and den(j-1)'s
                # issue, giving ACT a full tile of slack to deliver exp(j-1) —
                # den never head-of-line-blocks the DVE queue.  stage_c(j-2)
                # last, so its ACT chain sits behind the latency-critical
                # copies/exp in the in-order ACT queue.
                pend.append(stage_a(it, xt_text, xt_img, slice(j * BT, (j + 1) * BT)))
                if len(pend) >= 2:
                    stage_b(pend[-2])
                if len(pend) >= 3:
                    stage_c(pend[-3])
        stage_c(pend[-2])
        stage_b(pend[-1])
        stage_c(pend[-1])


@functools.lru_cache(maxsize=2)
def _built(b_loc: int):
    return build_bass(b_loc)


def _dr_pack(a_t: np.ndarray, nchunks2: int, width: int) -> np.ndarray:
    """[d_in, F] -> DoubleRow layout [128, nchunks2, 2, F] with
    d_in = c*256 + i*128 + p."""
    return np.ascontiguousarray(
        a_t.reshape(nchunks2, 2, 128, width).transpose(2, 0, 1, 3)
    )


def _shard_inputs(inputs: dict, b_loc: int, n_cores: int):
    f32 = lambda a: np.asarray(a, dtype=np.float32)
    text = f32(inputs["text_features"])
    image = f32(inputs["image_features"])
    f8 = lambda a: np.ascontiguousarray(a).astype(NP_F8)
    wq8 = _dr_pack(np.asarray(inputs["Wq"], np.float32).T * S_W, NTC2, TEXT_DIM).astype(NP_F8)
    wk8 = _dr_pack(np.asarray(inputs["Wk"], np.float32).T * S_W, NIC2, TEXT_DIM).astype(NP_F8)
    # V output features permuted to the [d][g] attend layout
    wv8 = _dr_pack(
        (np.asarray(inputs["Wv"], np.float32).T * S_W)[:, V_PERM], NIC2, TEXT_DIM
    ).astype(NP_F8)
    row16 = lambda a: np.asarray(a, np.float32).reshape(1, -1).astype(np.float16)
    bq, bk = row16(np.asarray(inputs["bq"]) * S_W), row16(np.asarray(inputs["bk"]) * S_W)
    bv = row16(np.asarray(inputs["bv"])[V_PERM] * S_W)

    in_maps = []
    for c in range(n_cores):
        sl = slice(c * b_loc, (c + 1) * b_loc)
        in_maps.append(
            {
                "text_t8": _dr_pack(text[sl].T, NTC2, b_loc).astype(NP_F8),
                "image_t8": _dr_pack(image[sl].T, NIC2, b_loc).astype(NP_F8),
                "text16": np.ascontiguousarray(text[sl]).astype(np.float16),
                "wq8": wq8,
                "wk8": wk8,
                "wv8": wv8,
                "bq": bq,
                "bk": bk,
                "bv": bv,
            }
        )
    return in_maps


def kernel(**inputs) -> np.ndarray:
    nc = _built(B_LOC)
    in_maps = _shard_inputs(inputs, B_LOC, N_CORES)
    res = bass_utils.run_bass_kernel_spmd(nc, in_maps, core_ids=list(range(N_CORES)))
    out = np.concatenate([np.asarray(r["y"], np.float32) for r in res.results], axis=0)
    gamma = np.asarray(inputs["gamma"], np.float32)
    beta = np.asarray(inputs["beta"], np.float32)
    if not (np.all(gamma == 1.0) and np.all(beta == 0.0)):
        out = out * gamma + beta  # general case; identity for this problem
    return out


# revision 25
# speedup vs baseline: 1.2251x; 1.2251x over previous
"""
MultiHeadCrossAttention Trainium2 kernel (Bass/Tile), data-parallel over batch
on 8 NeuronCores.

Reference computation (per batch row b):
    Q = text @ Wq.T + bq          [B, 1024] -> [B, 8, 128]
    K = image @ Wk.T + bk         [B, 2048] -> [B, 8, 128]
    V = image @ Wv.T + bv         [B, 2048] -> [B, 8, 128]
    scores[b,h,g] = Q[b,h,:].K[b,g,:] / sqrt(128)
    attn = softmax_g(scores)
    attended[b,h,:] = sum_g attn[b,h,g] V[b,g,:]
    y = LayerNorm(text + attended) * gamma + beta

v2 design (per core, B_loc = 2048 batch rows):
  - Projections on the PE in fp8e4 with DoubleRow perf mode (2 contraction
    k-tiles of 128 packed per matmul -> 0.5 cycles/row).  Host pre-casts
    X^T slabs (natural scale) and W^T (scaled x128) to e4m3; biases ride a
    K=1 fp16 matmul pre-scaled x128; the PSUM->SBUF ACT copy applies 1/128,
    restoring natural units in fp16.
  - Attention entirely in batch-on-partition layout on the DVE: broadcast
    product + dense binary add-tree for the d-reduction (scores) and the
    g-reduction (attend).  These are the critical-path ops (TT runs at
    2 elem/cycle fp16; nothing else on the chip multiplies tensors faster).
  - LayerNorm stats via ACT accumulate (Square+accum, Identity+accum), the
    rsqrt via Ln/Exp (one ACT table), normalize+shift as a single ACT op
    writing the fp16 output tile directly.  gamma/beta are identity in this
    problem (ones/zeros) and are applied on host in the (never-taken)
    general case, so the device kernel skips them.
"""

import functools
import sys

import numpy as np

sys.path.insert(0, "/opt/trn_rl_repo")

import concourse.bass as bass  # noqa: E402
import concourse.tile as tile  # noqa: E402
from concourse import bacc, bass_utils, mybir  # noqa: E402

import ml_dtypes  # noqa: E402


def _patch_act_tables():
    """Force every activation we use (Exp/Ln/Square/Copy/Identity) to resolve
    to the one table set that holds them all (natural_log_exp_and_others), so
    bacc emits a single ACT table load instead of thrashing (1.28us/swap)."""
    import concourse.hw_specs as hw_specs

    orig = hw_specs.get_activation_tables
    if getattr(orig, "_mhca_patched", False):
        return

    A = mybir.ActivationFunctionType
    KEEP = "natural_log_exp_and_others"

    @functools.cache
    def patched(arch):
        tabs = {k: set(v) for k, v in orig(arch).items()}
        for k, s in tabs.items():
            if k != KEEP:
                for f in (A.Exp, A.Ln, A.Square, A.Copy, A.Identity):
                    s.discard(f)
        return tabs

    patched._mhca_patched = True
    hw_specs.get_activation_tables = patched
    import concourse.bass_interp as _bi

    _bi.get_activation_tables = patched
    bacc.get_activation_tables = patched


_patch_act_tables()

# Problem constants (hardcoded per contest contract)
B = 16384
N_CORES = 8
B_LOC = B // N_CORES  # 2048
TEXT_DIM = 1024
IMAGE_DIM = 2048
H = 8
HD = 128
NTC2 = TEXT_DIM // 256  # 4 DoubleRow k-chunks (256 each)
NIC2 = IMAGE_DIM // 256  # 8 DoubleRow k-chunks

BT = 128  # batch tile (partition dim)
PHASE = 2  # batch tiles per X^T slab load

F8 = mybir.dt.float8e4
F16 = mybir.dt.float16
F32 = mybir.dt.float32
NP_F8 = ml_dtypes.float8_e4m3

S_W = 128.0  # fp8 weight pre-scale (power of 2; undone in the psum copy)

INV_SQRT_HD = 1.0 / np.sqrt(128.0)

# V feature permutation: f' = d*8 + g for original f = g*128 + d, i.e. V is
# stored with the 8 head values of each hidden position adjacent, so the
# attend product / g-reduction reads contiguous 8-element runs.
_d, _g = np.meshgrid(np.arange(128), np.arange(8), indexing="ij")
V_PERM = (_g * 128 + _d).reshape(-1)  # V_PERM[f'] = original f

DR = mybir.MatmulPerfMode.DoubleRow


def build_bass(b_loc: int = B_LOC) -> bass.Bass:
    nt = b_loc // BT
    phase = min(PHASE, nt)
    bw = phase * BT

    nc = bacc.Bacc(trn_type="TRN2", debug=False, name="mhca_dp", num_swdge_queues=4)

    # ---- DRAM I/O ----
    # X^T slabs in DoubleRow layout: d_in = c*256 + i*128 + p
    text_t8 = nc.dram_tensor("text_t8", [128, NTC2, 2, b_loc], F8, kind="ExternalInput")
    image_t8 = nc.dram_tensor("image_t8", [128, NIC2, 2, b_loc], F8, kind="ExternalInput")
    text16 = nc.dram_tensor("text16", [b_loc, TEXT_DIM], F16, kind="ExternalInput")
    wq8 = nc.dram_tensor("wq8", [128, NTC2, 2, TEXT_DIM], F8, kind="ExternalInput")
    wk8 = nc.dram_tensor("wk8", [128, NIC2, 2, TEXT_DIM], F8, kind="ExternalInput")
    wv8 = nc.dram_tensor("wv8", [128, NIC2, 2, TEXT_DIM], F8, kind="ExternalInput")
    bq = nc.dram_tensor("bq", [1, TEXT_DIM], F16, kind="ExternalInput")
    bk = nc.dram_tensor("bk", [1, TEXT_DIM], F16, kind="ExternalInput")
    bv = nc.dram_tensor("bv", [1, TEXT_DIM], F16, kind="ExternalInput")
    y = nc.dram_tensor("y", [b_loc, TEXT_DIM], F16, kind="ExternalOutput")

    with tile.TileContext(nc) as tc:
        _body(nc, tc, locals(), nt=nt, phase=phase, bw=bw)
    nc.compile()
    return nc


def _ap(t: bass.AP, dims) -> bass.AP:
    """Raw AP on an SBUF tile: keep its partition dim, custom free dims."""
    return bass.AP(tensor=t.tensor, offset=t.offset, ap=[list(t.ap[0])] + [list(d) for d in dims])


def _body(nc: bass.Bass, tc: tile.TileContext, io: dict, *, nt: int, phase: int, bw: int):
    text_t8, image_t8, text16 = io["text_t8"], io["image_t8"], io["text16"]
    wq8, wk8, wv8 = io["wq8"], io["wk8"], io["wv8"]
    bq, bk, bv = io["bq"], io["bk"], io["bv"]
    y = io["y"]

    import contextlib

    ctx = contextlib.ExitStack()
    with ctx:
        consts = ctx.enter_context(tc.tile_pool(name="consts", bufs=1))
        slabs = ctx.enter_context(tc.tile_pool(name="slabs", bufs=3))
        qkv = ctx.enter_context(tc.tile_pool(name="qkv", bufs=3))
        work = ctx.enter_context(tc.tile_pool(name="work", bufs=2))
        prods = ctx.enter_context(tc.tile_pool(name="prods", bufs=2))
        scr2p = ctx.enter_context(tc.tile_pool(name="scr2p", bufs=2))
        outs = ctx.enter_context(tc.tile_pool(name="outs", bufs=2))
        small = ctx.enter_context(tc.tile_pool(name="small", bufs=3))
        psum = ctx.enter_context(tc.tile_pool(name="psum", bufs=4, space="PSUM"))

        # ---- constants / weights (fp8, host-prepared) ----
        w8_q = consts.tile([128, NTC2, 2, TEXT_DIM], F8)
        w8_k = consts.tile([128, NIC2, 2, TEXT_DIM], F8)
        w8_v = consts.tile([128, NIC2, 2, TEXT_DIM], F8)
        # weight loads spread over the two HWDGE channels (sync / scalar),
        # ordered so tile 0's consumption order (Q -> K -> V) is also the
        # landing order.  Keeping them off the gpsimd SWDGE rings matters:
        # those also carry the X^T slabs, and a 2MB weight load queued behind
        # a slab stalls the first tiles by tens of us.
        wload = [
            (nc.sync, w8_q, wq8, 0, 4),
            (nc.scalar, w8_k, wk8, 0, 4),
            (nc.gpsimd, w8_k, wk8, 4, 4),
            (nc.gpsimd, w8_v, wv8, 0, 4),
            (nc.gpsimd, w8_v, wv8, 4, 4),
        ]

        b16 = consts.tile([1, 3, TEXT_DIM], F16)
        nc.gpsimd.dma_start(out=b16[:, 0, :], in_=bq[:])
        nc.gpsimd.dma_start(out=b16[:, 1, :], in_=bk[:])
        nc.gpsimd.dma_start(out=b16[:, 2, :], in_=bv[:])

        ones16 = consts.tile([1, 128], F16)
        nc.vector.memset(ones16, 1.0)

        eps_sb = consts.tile([128, 1], F32)
        nc.vector.memset(eps_sb, 1e-5)
        sq16 = consts.tile([128, TEXT_DIM], F16)  # ACT-accum scratch output

        # ---------------- 3-stage software pipeline ----------------
        # stage A (iter j):   projections + psum copies + scores + exp
        # stage B (iter j+1): softmax weights + attend + residual add
        # stage C (iter j+2): layernorm + store
        # This keeps the DVE FIFO free of head-of-line stalls: every DVE op
        # emitted only depends on work issued at least one iteration earlier.

        def project(xt, w8, nchunks2, bias_idx, bs):
            pt = psum.tile([128, TEXT_DIM], F32, tag="psum")
            for f in range(2):
                half = pt[:, f * 512 : (f + 1) * 512]
                for c in range(nchunks2):
                    nc.tensor.matmul(
                        half,
                        lhsT=xt[:, c, :, bs],
                        rhs=w8[:, c, :, f * 512 : (f + 1) * 512],
                        start=(c == 0),
                        stop=False,
                        perf_mode=DR,
                    )
                nc.tensor.matmul(
                    half,
                    lhsT=ones16,
                    rhs=b16[:, bias_idx, f * 512 : (f + 1) * 512],
                    start=False,
                    stop=True,
                )
            return pt

        def stage_a(it, xt_text, xt_img, bs):
            row0 = it * BT
            text_sb = work.tile([128, TEXT_DIM], F16, tag="text_sb")
            nc.sync.dma_start(out=text_sb, in_=text16[row0 : row0 + BT, :])

            qp = project(xt_text, w8_q, NTC2, 0, bs)
            kp = project(xt_img, w8_k, NIC2, 1, bs)
            vp = project(xt_img, w8_v, NIC2, 2, bs)

            # PSUM -> SBUF fp16 copies (ACT), undoing the x128 weight scale.
            # Q/K first — the DVE scores product needs them next; the V copy
            # is emitted after the exp (below) so the softmax-denominator
            # dependency is never queued behind it.  Wv/bv are host-permuted
            # to the [d][g] attend layout, so all copies are contiguous.
            q16 = qkv.tile([128, TEXT_DIM], F16, tag="q16")
            k16 = qkv.tile([128, TEXT_DIM], F16, tag="k16")
            vt16 = qkv.tile([128, TEXT_DIM], F16, tag="vt16")
            nc.scalar.mul(q16, qp, 1.0 / S_W)
            nc.scalar.mul(k16, kp, 1.0 / S_W)

            # scores: prod[b, h, g, d] = Q[b,h,d] * K[b,g,d]
            prod = prods.tile([128, H * H * HD], F16, tag="prod")
            scr2 = scr2p.tile([128, H * H * HD // 2], F16, tag="scr2")
            nc.vector.tensor_tensor(
                out=prod[:].rearrange("p (h g d) -> p h g d", h=H, g=H),
                in0=_ap(q16, [[128, 8], [0, 8], [1, 128]]),
                in1=_ap(k16, [[0, 8], [128, 8], [1, 128]]),
                op=mybir.AluOpType.mult,
            )
            # d-reduction: binary TT-add tree with dense (compacted) outputs
            # ping-ponging between prod and scr2 — sparse in-place outputs hit
            # a DVE slow path — then one tensor_reduce of the remaining 8.
            cur, nxt = prod, scr2
            d = HD
            while d > 8:
                nc.vector.tensor_tensor(
                    out=_ap(nxt, [[d // 2, H * H], [1, d // 2]]),
                    in0=_ap(cur, [[d, H * H], [1, d // 2]]),
                    in1=bass.AP(tensor=cur.tensor, offset=cur.offset + d // 2,
                                ap=[list(cur.ap[0]), [d, H * H], [1, d // 2]]),
                    op=mybir.AluOpType.add,
                )
                cur, nxt = nxt, cur
                d //= 2
            s16 = small.tile([128, H * H], F16, tag="s16")
            with nc.allow_low_precision("fp16 scores; DVE ALU accumulates fp32"):
                nc.vector.tensor_reduce(
                    out=s16,
                    in_=_ap(cur, [[8, H * H], [1, 8]]),
                    axis=mybir.AxisListType.X,
                    op=mybir.AluOpType.add,
                )
            e16 = small.tile([128, H * H], F16, tag="e16")
            nc.scalar.activation(
                out=e16, in_=s16,
                func=mybir.ActivationFunctionType.Exp,
                scale=float(INV_SQRT_HD),
            )
            # V copy after the exp: stage_b consumes it one iteration later
            nc.scalar.mul(vt16, vp, 1.0 / S_W)
            return dict(it=it, text_sb=text_sb, vt16=vt16, e16=e16, prod=prod, scr2=scr2)

        def stage_b(t):
            e16, vt16, prod, scr2 = t["e16"], t["vt16"], t["prod"], t["scr2"]
            den = small.tile([128, H], F32, tag="den")
            nc.vector.tensor_reduce(
                out=den,
                in_=e16[:].rearrange("p (h g) -> p h g", h=H),
                axis=mybir.AxisListType.X,
                op=mybir.AluOpType.add,
            )
            rden = small.tile([128, H], F32, tag="rden")
            nc.vector.reciprocal(out=rden, in_=den)  # = 1 / sum_g exp
            a16 = small.tile([128, H * H], F16, tag="a16")
            nc.vector.tensor_tensor(
                out=a16[:].rearrange("p (h g) -> p h g", h=H),
                in0=e16[:].rearrange("p (h g) -> p h g", h=H),
                in1=_ap(rden, [[1, 8], [0, 8]]),
                op=mybir.AluOpType.mult,
            )
            # attend, A/B-split over g so every reduction stream is long-run:
            #   prodA[b, h, d, g<4]  = Vperm[b, d*8+g]   * A[b,h,g]
            #   prodB[b, h, d, g'<4] = Vperm[b, d*8+4+g'] * A[b,h,4+g']
            # (V stream first: long contiguous runs on the src0 port)
            half = H * HD * 4  # 4096
            for off in (0, 4):
                nc.vector.tensor_tensor(
                    out=bass.AP(tensor=prod.tensor, offset=prod.offset + (off // 4) * half,
                                ap=[list(prod.ap[0]), [4, H * HD], [1, 4]]),
                    in0=bass.AP(tensor=vt16.tensor, offset=vt16.offset + off,
                                ap=[list(vt16.ap[0]), [0, 8], [8, 128], [1, 4]]),
                    in1=bass.AP(tensor=a16.tensor, offset=a16.offset + off,
                                ap=[list(a16.ap[0]), [8, 8], [0, 128], [1, 4]]),
                    op=mybir.AluOpType.mult,
                )
            # g-reduction: L1 = A-half + B-half (both fully contiguous)
            nc.vector.tensor_tensor(
                out=_ap(scr2, [[1, half]]),
                in0=_ap(prod, [[1, half]]),
                in1=bass.AP(tensor=prod.tensor, offset=prod.offset + half,
                            ap=[list(prod.ap[0]), [1, half]]),
                op=mybir.AluOpType.add,
            )
            # L2: pairs (g, g+2) within each [h,d] group of 4
            nc.vector.tensor_tensor(
                out=_ap(prod, [[2, H * HD], [1, 2]]),
                in0=_ap(scr2, [[4, H * HD], [1, 2]]),
                in1=bass.AP(tensor=scr2.tensor, offset=scr2.offset + 2,
                            ap=[list(scr2.ap[0]), [4, H * HD], [1, 2]]),
                op=mybir.AluOpType.add,
            )
            att16 = work.tile([128, TEXT_DIM], F16, tag="att16")
            nc.vector.tensor_tensor(
                out=att16,
                in0=_ap(prod, [[2, H * HD]]),
                in1=bass.AP(tensor=prod.tensor, offset=prod.offset + 1,
                            ap=[list(prod.ap[0]), [2, H * HD]]),
                op=mybir.AluOpType.add,
            )
            # residual add stays on the DVE: contiguous fp16 at 2 elem/cycle
            # beats the gpsimd software add by ~10x and unblocks the pipeline.
            x = work.tile([128, TEXT_DIM], F16, tag="x")
            nc.vector.tensor_tensor(
                out=x, in0=t["text_sb"], in1=att16, op=mybir.AluOpType.add
            )
            t["x"] = x

        def stage_c(t):
            x = t["x"]
            row0 = t["it"] * BT
            A = mybir.ActivationFunctionType
            # LN stats on ACT: ssq = sum x^2, sx = sum x (sq16 is a dump tile)
            ssq = small.tile([128, 1], F32, tag="ssq")
            sx = small.tile([128, 1], F32, tag="sx")
            nc.scalar.activation(out=sq16, in_=x, func=A.Square, accum_out=ssq)
            nc.scalar.activation(out=sq16, in_=x, func=A.Identity, accum_out=sx)
            # [128,1] scalar math on the (otherwise idle) gpsimd:
            # mu = sx/D ; m2 = -mu^2 ; varp = ssq/D + eps - mu^2
            mu = small.tile([128, 1], F32, tag="mu")
            nc.gpsimd.tensor_scalar(
                out=mu, in0=sx, scalar1=1.0 / TEXT_DIM, scalar2=1.0,
                op0=mybir.AluOpType.mult, op1=mybir.AluOpType.mult,
            )
            m2 = small.tile([128, 1], F32, tag="m2")
            nc.gpsimd.tensor_scalar(
                out=m2, in0=mu, scalar1=mu, scalar2=-1.0,
                op0=mybir.AluOpType.mult, op1=mybir.AluOpType.mult,
            )
            varp = small.tile([128, 1], F32, tag="varp")
            nc.gpsimd.tensor_scalar(
                out=varp, in0=ssq, scalar1=1.0 / TEXT_DIM, scalar2=m2,
                op0=mybir.AluOpType.mult, op1=mybir.AluOpType.add,
            )
            # rs = 1/sqrt(var+eps) = exp(-0.5*ln(var+eps)); Ln and Exp live in
            # the same ACT table (natural_log_exp_and_others), Sqrt does not.
            # eps rides the Ln bias.
            lnv = small.tile([128, 1], F32, tag="lnv")
            nc.scalar.activation(out=lnv, in_=varp, func=A.Ln, bias=eps_sb)
            rs = small.tile([128, 1], F32, tag="rs")
            nc.scalar.activation(out=rs, in_=lnv, func=A.Exp, scale=-0.5)
            nmr = small.tile([128, 1], F32, tag="nmr")
            nc.gpsimd.tensor_scalar(
                out=nmr, in0=mu, scalar1=rs, scalar2=-1.0,
                op0=mybir.AluOpType.mult, op1=mybir.AluOpType.mult,
            )
            # y = x*rs - mu*rs on ACT — keeps the wall engine (DVE) clean;
            # this sits at the end of ACT's queue where nothing urgent follows.
            # (gamma/beta are identity; host applies the general case)
            y16 = outs.tile([128, TEXT_DIM], F16, tag="y16")
            nc.scalar.activation(out=y16, in_=x, func=A.Identity, scale=rs, bias=nmr)
            nc.gpsimd.dma_start(out=y[row0 : row0 + BT, :], in_=y16)

        def load_slab(p):
            b0 = p * bw
            xt_t = slabs.tile([128, NTC2, 2, bw], F8, tag="xt_text")
            xt_i = slabs.tile([128, NIC2, 2, bw], F8, tag="xt_img")
            nc.gpsimd.dma_start(out=xt_t, in_=text_t8[:, :, :, b0 : b0 + bw])
            nc.gpsimd.dma_start(out=xt_i, in_=image_t8[:, :, :, b0 : b0 + bw])
            return xt_t, xt_i

        pend = []
        n_phases = (nt + phase - 1) // phase
        slab_next = load_slab(0)
        for ph in range(n_phases):
            xt_text, xt_img = slab_next
            if ph == 0:
                # weights after the first slab so tile 0 lhsT lands first
                for eng, w8, wr, c0, cn in wload:
                    eng.dma_start(
                        out=w8[:, c0 : c0 + cn], in_=wr[:, c0 : c0 + cn]
                    )
            if ph + 1 < n_phases:
                # prefetch the next phase's slab before this phase's tiles so
                # a phase boundary never stalls on the X^T load
                slab_next = load_slab(ph + 1)

            for j in range(phase):
                it = ph * phase + j
                if it >= nt:
                    break
                # stage_a(j) BEFORE stage_b(j-1): the DVE then grinds through
                # prod1/tree(j) between s16(j-1)'s completion and den(j-1)'s
                # issue, giving ACT a full tile of slack to deliver exp(j-1) —
                # den never head-of-line-blocks the DVE queue.  stage_c(j-2)
                # last, so its ACT chain sits behind the latency-critical
                # copies/exp in the in-order ACT queue.
                pend.append(stage_a(it, xt_text, xt_img, slice(j * BT, (j + 1) * BT)))
                if len(pend) >= 2:
                    stage_b(pend[-2])
                if len(pend) >= 3:
                    stage_c(pend[-3])
        stage_c(pend[-2])
        stage_b(pend[-1])
        stage_c(pend[-1])


@functools.lru_cache(maxsize=2)
def _built(b_loc: int):
    return build_bass(b_loc)


def _dr_pack(a_t: np.ndarray, nchunks2: int, width: int) -> np.ndarray:
    """[d_in, F] -> DoubleRow layout [128, nchunks2, 2, F] with
    d_in = c*256 + i*128 + p."""
    return np.ascontiguousarray(
        a_t.reshape(nchunks2, 2, 128, width).transpose(2, 0, 1, 3)
    )


def _shard_inputs(inputs: dict, b_loc: int, n_cores: int):
    f32 = lambda a: np.asarray(a, dtype=np.float32)
    text = f32(inputs["text_features"])
    image = f32(inputs["image_features"])
    f8 = lambda a: np.ascontiguousarray(a).astype(NP_F8)
    wq8 = _dr_pack(np.asarray(inputs["Wq"], np.float32).T * S_W, NTC2, TEXT_DIM).astype(NP_F8)
    wk8 = _dr_pack(np.asarray(inputs["Wk"], np.float32).T * S_W, NIC2, TEXT_DIM).astype(NP_F8)
    # V output features permuted to the [d][g] attend layout
    wv8 = _dr_pack(
        (np.asarray(inputs["Wv"], np.float32).T * S_W)[:, V_PERM], NIC2, TEXT_DIM
    ).astype(NP_F8)
    row16 = lambda a: np.asarray(a, np.float32).reshape(1, -1).astype(np.float16)
    bq, bk = row16(np.asarray(inputs["bq"]) * S_W), row16(np.asarray(inputs["bk"]) * S_W)
    bv = row16(np.asarray(inputs["bv"])[V_PERM] * S_W)

    in_maps = []
    for c in range(n_cores):
        sl = slice(c * b_loc, (c + 1) * b_loc)
        in_maps.append(
            {
                "text_t8": _dr_pack(text[sl].T, NTC2, b_loc).astype(NP_F8),
                "image_t8": _dr_pack(image[sl].T, NIC2, b_loc).astype(NP_F8),
                "text16": np.ascontiguousarray(text[sl]).astype(np.float16),
                "wq8": wq8,
                "wk8": wk8,
                "wv8": wv8,
                "bq": bq,
                "bk": bk,
                "bv": bv,
            }
        )
    return in_maps


def kernel(**inputs) -> np.ndarray:
    nc = _built(B_LOC)
    in_maps = _shard_inputs(inputs, B_LOC, N_CORES)
    res = bass_utils.run_bass_kernel_spmd(nc, in_maps, core_ids=list(range(N_CORES)))
    out = np.concatenate([np.asarray(r["y"], np.float32) for r in res.results], axis=0)
    gamma = np.asarray(inputs["gamma"], np.float32)
    beta = np.asarray(inputs["beta"], np.float32)
    if not (np.all(gamma == 1.0) and np.all(beta == 0.0)):
        out = out * gamma + beta  # general case; identity for this problem
    return out
